# revision 5
# baseline (speedup 1.0000x reference)
"""Bahdanau additive attention kernel for Trainium2 (8 NeuronCores).

Problem shapes (hardcoded): B=4, Q=256, V=2048, H=512, U=128, fp32.

reference:
    pq = queries @ w1                  # [B,Q,U]
    pv = values  @ w2                  # [B,V,U]
    scores[b,q,v] = sum_u tanh(pq[b,q,u] + pv[b,v,u]) * v[u]
    attn = softmax(scores, axis=-1)
    out  = attn @ values               # [B,Q,H]

Sharding: 8 cores = 4 batches x 2 query-halves. Each core handles a full
softmax over V for its [128, H] query slice -> no collectives needed.

Per-core dataflow:
  - pqT [U=128, Qloc=128] and pvT [U=128, V=2048] built on the PE
    (transposes via identity matmul, then w1/w2 projections).
  - loop over q (128 iters):
      ACT: t_q = tanh(pvT + pqT[:,q] as per-partition bias)  [128,2048] fp16
      PE : scores[q,:] += v^T t_q via a shifted-window stationary
           (v embedded at column q of a [128,256] zero pad -> lhsT window
           [128,128] has v in column q, so the matvec result lands in PSUM
           partition q; all other partitions accumulate zeros)
  - softmax over the free axis (DVE max/sum, ACT exp, DVE reciprocal)
  - out = (e^T)^T ... : 16 PE transposes of e, then 16 accumulating fp32
    matmuls against the resident values tiles, then scale rows by 1/sum.
"""

from contextlib import ExitStack

import numpy as np

import concourse.bacc as bacc
import concourse.bass as bass
import concourse.tile as tile
from concourse import mybir
from concourse.masks import make_identity

B, Q, V, H, U = 4, 256, 2048, 512, 128
QL = Q // 2            # per-core queries
VT = V // 128          # 16 value tiles
HT = H // 128          # 4 hidden tiles
NB = V // 512          # 4 psum bank chunks of the scores row

F32 = mybir.dt.float32
F16 = mybir.dt.float16


def build_nc(t_dtype=F16):
    nc = bacc.Bacc("TRN2", target_bir_lowering=False, debug=False)
    q_ext = nc.declare_dram_parameter("q_shard", [QL, H], F32, isOutput=False)
    vals_ext = nc.declare_dram_parameter("vals", [VT, 128, H], F32, isOutput=False)
    w1_ext = nc.declare_dram_parameter("w1", [HT, 128, U], F32, isOutput=False)
    w2_ext = nc.declare_dram_parameter("w2", [HT, 128, U], F32, isOutput=False)
    v_ext = nc.declare_dram_parameter("v", [U, 1], F32, isOutput=False)
    out_ext = nc.declare_dram_parameter("out", [QL, H], F32, isOutput=True)

    with tile.TileContext(nc) as tc, ExitStack() as ctx:
        singles = ctx.enter_context(tc.tile_pool(name="singles", bufs=1))
        work = ctx.enter_context(tc.tile_pool(name="work", bufs=3))
        tpool = ctx.enter_context(tc.tile_pool(name="tanh", bufs=3))
        ps = ctx.enter_context(tc.tile_pool(name="ps", bufs=3, space="PSUM"))
        ps_big = ctx.enter_context(tc.tile_pool(name="ps_big", bufs=1, space="PSUM"))

        identity = singles.tile([128, 128], F32)
        make_identity(nc, identity)

        # --- constants -------------------------------------------------
        sb_w1 = singles.tile([128, HT, U], F32)
        sb_w2 = singles.tile([128, HT, U], F32)
        for ht in range(HT):
            nc.sync.dma_start(out=sb_w1[:, ht, :], in_=w1_ext[ht])
            nc.sync.dma_start(out=sb_w2[:, ht, :], in_=w2_ext[ht])
        sb_v = singles.tile([128, 1], F32)
        nc.sync.dma_start(out=sb_v, in_=v_ext[:])
        # v embedded at column 128 of a zero pad; window [:, 128-q:256-q]
        # puts v at window-column q.
        sb_vpad = singles.tile([128, 256], t_dtype)
        nc.vector.memset(sb_vpad, 0.0)
        nc.vector.tensor_copy(out=sb_vpad[:, 128:129], in_=sb_v)

        # --- queries -> pqT [u, q] ------------------------------------
        sb_q = work.tile([128, H], F32)
        nc.sync.dma_start(out=sb_q, in_=q_ext[:])
        sb_qT = singles.tile([128, HT, 128], F32)
        for ht in range(HT):
            ps_tr = ps.tile([128, 512], F32, tag="ps_scratch")
            nc.tensor.transpose(ps_tr[:, :128], sb_q[:, ht * 128:(ht + 1) * 128], identity)
            nc.vector.tensor_copy(out=sb_qT[:, ht, :], in_=ps_tr[:, :128])
        sb_pqT = singles.tile([128, QL], F32)
        ps_pq = ps.tile([128, 512], F32, tag="ps_scratch")
        for ht in range(HT):
            nc.tensor.matmul(
                ps_pq[:, :QL], lhsT=sb_w1[:, ht, :], rhs=sb_qT[:, ht, :],
                start=(ht == 0), stop=(ht == HT - 1),
            )
        nc.vector.tensor_copy(out=sb_pqT, in_=ps_pq[:, :QL])

        # --- values -> sbuf (resident) and pvT [u, v] ------------------
        sb_vals = singles.tile([128, VT, H], F32)
        sb_pvT = singles.tile([128, V], F32)
        for vt in range(VT):
            nc.sync.dma_start(out=sb_vals[:, vt, :], in_=vals_ext[vt])
        for vt in range(VT):
            sb_vT = work.tile([128, HT, 128], F32)
            for ht in range(HT):
                ps_tr = ps.tile([128, 512], F32, tag="ps_scratch")
                nc.tensor.transpose(
                    ps_tr[:, :128], sb_vals[:, vt, ht * 128:(ht + 1) * 128], identity)
                nc.vector.tensor_copy(out=sb_vT[:, ht, :], in_=ps_tr[:, :128])
            ps_pv = ps.tile([128, 512], F32, tag="ps_scratch")
            for ht in range(HT):
                nc.tensor.matmul(
                    ps_pv[:, :128], lhsT=sb_w2[:, ht, :], rhs=sb_vT[:, ht, :],
                    start=(ht == 0), stop=(ht == HT - 1),
                )
            nc.vector.tensor_copy(out=sb_pvT[:, vt * 128:(vt + 1) * 128], in_=ps_pv[:, :128])

        # --- main loop: tanh + matvec reduction over u -----------------
        psum_scores = ps_big.tile([128, V], F32)
        for q in range(QL):
            t_t = tpool.tile([128, V], t_dtype, tag="t")
            nc.scalar.activation(
                out=t_t, in_=sb_pvT,
                func=mybir.ActivationFunctionType.Tanh,
                bias=sb_pqT[:, q:q + 1], scale=1.0,
            )
            for nb in range(NB):
                nc.tensor.matmul(
                    psum_scores[:, nb * 512:(nb + 1) * 512],
                    lhsT=sb_vpad[:, 128 - q:256 - q],
                    rhs=t_t[:, nb * 512:(nb + 1) * 512],
                    start=(q == 0), stop=(q == QL - 1),
                )

        # --- softmax ---------------------------------------------------
        sb_max = work.tile([128, 1], F32)
        nc.vector.tensor_reduce(
            out=sb_max, in_=psum_scores, axis=mybir.AxisListType.X,
            op=mybir.AluOpType.max,
        )
        sb_negmax = work.tile([128, 1], F32)
        nc.vector.tensor_scalar_mul(sb_negmax, sb_max, -1.0)
        sb_e = singles.tile([128, V], F32)
        nc.scalar.activation(
            out=sb_e, in_=psum_scores,
            func=mybir.ActivationFunctionType.Exp,
            bias=sb_negmax, scale=1.0,
        )
        sb_sum = work.tile([128, 1], F32)
        nc.vector.tensor_reduce(
            out=sb_sum, in_=sb_e, axis=mybir.AxisListType.X,
            op=mybir.AluOpType.add,
        )
        sb_rsum = work.tile([128, 1], F32)
        nc.vector.reciprocal(sb_rsum, sb_sum)

        # --- out = diag(1/sum) @ e @ values ----------------------------
        sb_eT = singles.tile([128, VT, 128], F32)
        for vt in range(VT):
            ps_tr = ps.tile([128, 512], F32, tag="ps_scratch")
            nc.tensor.transpose(ps_tr[:, :128], sb_e[:, vt * 128:(vt + 1) * 128], identity)
            nc.vector.tensor_copy(out=sb_eT[:, vt, :], in_=ps_tr[:, :128])
        ps_out = ps.tile([128, 512], F32, tag="ps_scratch")
        for vt in range(VT):
            nc.tensor.matmul(
                ps_out, lhsT=sb_eT[:, vt, :], rhs=sb_vals[:, vt, :],
                start=(vt == 0), stop=(vt == VT - 1),
            )
        sb_out = work.tile([128, H], F32)
        nc.vector.tensor_scalar_mul(sb_out, ps_out, sb_rsum)
        nc.sync.dma_start(out=out_ext[:], in_=sb_out)

    nc.finalize()
    return nc


_NC_CACHE = {}


def _get_nc():
    if "nc" not in _NC_CACHE:
        _NC_CACHE["nc"] = build_nc()
    return _NC_CACHE["nc"]


def make_in_maps(queries, values, w1, w2, v):
    w1s = np.ascontiguousarray(w1, np.float32).reshape(HT, 128, U)
    w2s = np.ascontiguousarray(w2, np.float32).reshape(HT, 128, U)
    vs = np.ascontiguousarray(v, np.float32).reshape(U, 1)
    in_maps = []
    for c in range(8):
        b, qh = c // 2, c % 2
        in_maps.append({
            "q_shard": np.ascontiguousarray(
                queries[b, qh * QL:(qh + 1) * QL, :], np.float32),
            "vals": np.ascontiguousarray(values[b], np.float32).reshape(VT, 128, H),
            "w1": w1s, "w2": w2s, "v": vs,
        })
    return in_maps


def gather_out(results):
    out = np.empty((B, Q, H), np.float32)
    for c in range(8):
        b, qh = c // 2, c % 2
        out[b, qh * QL:(qh + 1) * QL, :] = results[c]["out"]
    return out


def kernel(queries, values, w1, w2, v):
    from concourse.bass_utils import run_bass_kernel_spmd

    nc = _get_nc()
    in_maps = make_in_maps(queries, values, w1, w2, v)
    res = run_bass_kernel_spmd(nc, in_maps, list(range(8)))
    return gather_out(res.results)
